# revision 44
# baseline (speedup 1.0000x reference)
"""AnatomicalAttention TRN2 kernel — 8-core data parallel.

Reference computation (B=4096, D=14, R=7, F=768):
    Q = X @ Wq.T + bq                 # [T, F], T = B*D tokens
    K = E @ Wk.T + bk                 # [R, F]
    V = E @ Wv.T + bv                 # [R, F]
    scores = (Q @ K.T) * scale * w    # [T, R]
    attn = softmax(scores, -1)
    attended = attn @ V               # [T, F]

Key rewrite: Q @ K.T = X @ (K @ Wq).T + bq @ K.T — K is only [7, 768], so the
[768,768] Q projection collapses into a rank-7 contraction with the tiny
precomputed matrix Keff = (K @ Wq) * scale * w.  Per 512-token super-tile:
    scoresT[r, t] = sum_f Keff[r, f] * X[t, f]       (PE, f on partitions)
    expT = exp(scoresT + bias) -> bf16               (ACT, PSUM -> SBUF)
    denom[t, g] = expT_g.T @ ones                    (PE, token-major)
    rcb = 1 / denom                                  (DVE, one batched recip)
    up = expT_g.T @ [V | I7]                         (PE: attended + exp cols)
    [attended | attn] = up * rcb[:, g]               (DVE, PSUM -> SBUF bf16)
The two phases are software-pipelined one super-tile apart so PE runs the
next tile's scores while ACT/DVE normalize the previous one; dummy "warm"
matmuls fill DMA-stall gaps so the PE HAM clock gate stays at 2.4 GHz.

Host passes X transposed ([F, T] feature-major) and in bf16 so DMA traffic is
halved and the contraction dim lands on SBUF partitions with contiguous
descriptors.  Outputs return as bf16 in blocked layouts (upcast/untangled on
host); attn accumulates on-chip and ships as one DMA.  Normalization is f32.
"""

from contextlib import ExitStack

import numpy as np
import ml_dtypes

import concourse.bass as bass
import concourse.tile as tile
from concourse import bacc, mybir
from concourse.bass import ds
from concourse.bass_utils import run_bass_kernel_spmd

B, D, R, F = 4096, 14, 7, 768
NCORES = 8
TOK = B * D                  # 57344 tokens
TPC = TOK // NCORES          # 7168 tokens per core
BLK = 512                    # tokens per DMA block
SUP = 512                    # tokens per score super-tile (fp32 moving-max)
SUB = 128                    # tokens per attended sub-tile (PE stationary max)
NBLK = TPC // BLK            # 7
FCH = F // 128               # 6 contraction chunks

_BF16 = mybir.dt.bfloat16
_F32 = mybir.dt.float32


def _build_nc() -> bass.Bass:
    nc = bacc.Bacc(trn_type="TRN2")

    xt = nc.declare_dram_parameter("xt", [F, TPC], _BF16, isOutput=False)
    keff = nc.declare_dram_parameter("keff", [128, FCH * R], _BF16, isOutput=False)
    # vaug = [V | I7]: attended matmul emits exp@V plus the token-major exp
    # columns used for the attn output, in one PE pass.
    vaug = nc.declare_dram_parameter("vaug", [R, F + R], _BF16, isOutput=False)
    ebias = nc.declare_dram_parameter("ebias", [R, 1], _F32, isOutput=False)
    onescol = nc.declare_dram_parameter("onescol", [R, 1], _BF16, isOutput=False)
    # blocked output layouts (host untangles): att[sup, p, j, f] = token sup*SUP+j*SUB+p
    NSUP = TPC // SUP
    GRP = SUP // SUB
    att = nc.declare_dram_parameter("att", [NSUP, 128, GRP, F], _BF16, isOutput=True)
    attn = nc.declare_dram_parameter("attn", [128, NSUP, GRP, R], _BF16, isOutput=True)

    with tile.TileContext(nc) as tc, ExitStack() as ctx:
        consts = ctx.enter_context(tc.tile_pool(name="consts", bufs=1))
        xpool = ctx.enter_context(tc.tile_pool(name="xpool", bufs=1))
        epool = ctx.enter_context(tc.tile_pool(name="epool", bufs=1))
        rfpool = ctx.enter_context(tc.tile_pool(name="rfpool", bufs=1))
        opool = ctx.enter_context(tc.tile_pool(name="opool", bufs=1))
        psa = ctx.enter_context(tc.tile_pool(name="psa", bufs=1, space="PSUM"))
        psd = ctx.enter_context(tc.tile_pool(name="psd", bufs=1, space="PSUM"))
        psu = ctx.enter_context(tc.tile_pool(name="psu", bufs=1, space="PSUM"))
        psw = ctx.enter_context(tc.tile_pool(name="psw", bufs=1, space="PSUM"))

        keff_sb = consts.tile([128, FCH, R], _BF16)
        nc.sync.dma_start(out=keff_sb, in_=keff[:].rearrange("p (c r) -> p c r", c=FCH))
        # small consts load via SWDGE (gpsimd) so the SP queue reaches the
        # first input-block DMAs immediately after keff
        vaug_sb = consts.tile([R, F + R], _BF16)
        nc.gpsimd.dma_start(out=vaug_sb, in_=vaug[:])
        ebias_sb = consts.tile([R, 1], _F32)
        nc.gpsimd.dma_start(out=ebias_sb, in_=ebias[:])
        onescol_sb = consts.tile([R, 1], _BF16)
        nc.gpsimd.dma_start(out=onescol_sb, in_=onescol[:])
        # PE keep-warm scratch: dummy matmuls fill DMA-stall gaps so the HAM
        # clock gate never sees an idle window and the PE stays at 2.4 GHz.
        warmsrc = consts.tile([128, 512], _BF16)
        nc.gpsimd.memset(warmsrc[:], 0.0)
        warm_ps = psw.tile([1, 512], _F32, tag="warm")
        # all attn output accumulates here (784B/partition); one DMA at the end
        attn_all = consts.tile([128, NSUP, GRP, R], _BF16)

        def warm(k):
            for _ in range(k):
                nc.tensor.matmul(warm_ps, warmsrc[:, 0:1], warmsrc, start=True, stop=True)

        xt_r = xt[:].rearrange("(c p) t -> p c t", p=128)

        def scores_phase(si):
            """Load + score matmuls.  Returns state."""
            warm(4)
            xts = xpool.tile([128, FCH, SUP], _BF16, tag=f"xts{si % 5}", name="xts")
            nc.sync.dma_start(out=xts, in_=xt_r[:, :, ds(si * SUP, SUP)])
            sct = psa.tile([R, SUP], _F32, tag=f"ps_sc{si % 2}")
            for c in range(FCH):
                nc.tensor.matmul(
                    sct,
                    keff_sb[:, c, :],
                    xts[:, c, :],
                    start=(c == 0),
                    stop=(c == FCH - 1),
                )
            return (si, sct)

        def exp_phase(state):
            """Emitted after the previous super's scales so ACT's in-order
            stream doesn't block them behind this exp."""
            si, sct = state
            expt = epool.tile([R, SUP], _BF16, tag=f"expt{si % 6}")
            nc.scalar.activation(
                expt, sct, mybir.ActivationFunctionType.Exp, bias=ebias_sb, scale=1.0
            )
            return (si, expt)

        def tail_phase(state):
            """Denominators, reciprocal, attended matmuls, scales, store."""
            si, expt = state
            dnt = psd.tile([128, GRP], _F32, tag="ps_dn")
            for sj in range(GRP):
                nc.tensor.matmul(
                    dnt[:, sj : sj + 1], expt[:, ds(sj * SUB, SUB)], onescol_sb
                )
            rcb = rfpool.tile([128, GRP], _F32, tag=f"rcb{si % 3}")
            nc.vector.reciprocal(rcb, dnt)

            atts = opool.tile([128, GRP, F + R], _BF16, tag=f"atts{si % 3}")
            # unnormalized attended (+ token-major exp columns), scaled by
            # 1/denom on the way out of PSUM; scales split across DVE/ACT.
            for sj in range(GRP):
                lhs = expt[:, ds(sj * SUB, SUB)]
                up = psu.tile([128, F + R], _F32, tag=f"up{sj % 2}")
                nc.tensor.matmul(up[:, 0:512], lhs, vaug_sb[:, 0:512])
                nc.tensor.matmul(up[:, 512 : F + R], lhs, vaug_sb[:, 512 : F + R])
                rc = rcb[:, sj : sj + 1]
                if sj < 2:
                    nc.vector.tensor_scalar_mul(atts[:, sj, :], up[:], rc)
                else:
                    nc.scalar.mul(atts[:, sj, :], up[:], mul=rc)
            # gather the attn columns into the whole-kernel accumulator (on
            # GPSIMD, which is otherwise idle); DMA'd once at kernel end.
            nc.gpsimd.tensor_copy(out=attn_all[:, si], in_=atts[:, :, F : F + R])
            nc.gpsimd.dma_start(out=att[si], in_=atts[:, :, 0:F])

        warm(14)  # pre-warm HAM while the first input DMA is in flight
        prev = None
        for si in range(NSUP):
            st = scores_phase(si)
            if prev is not None:
                tail_phase(prev)
            prev = exp_phase(st)
        tail_phase(prev)
        nc.gpsimd.dma_start(out=attn[:], in_=attn_all)

    nc.compile()
    return nc


_NC_CACHE: list = []


def _get_nc() -> bass.Bass:
    if not _NC_CACHE:
        _NC_CACHE.append(_build_nc())
    return _NC_CACHE[0]


def prepare_in_maps(features, region_embeddings, Wq, bq, Wk, bk, Wv, bv, region_weights):
    f32 = np.float32
    X = np.asarray(features, dtype=f32).reshape(TOK, F)
    E = np.asarray(region_embeddings, dtype=f32)
    Wq = np.asarray(Wq, dtype=f32)
    bq = np.asarray(bq, dtype=f32)
    Wk = np.asarray(Wk, dtype=f32)
    bk = np.asarray(bk, dtype=f32)
    Wv = np.asarray(Wv, dtype=f32)
    bv = np.asarray(bv, dtype=f32)
    w = np.asarray(region_weights, dtype=f32)

    scale = f32(F) ** -0.5
    K = E @ Wk.T + bk                      # [R, F]
    V = E @ Wv.T + bv                      # [R, F]
    cw = (scale * w).astype(f32)           # [R]
    keff2 = (K @ Wq) * cw[:, None]         # [R, F]
    sb2 = (K @ bq) * cw                    # [R]

    bf16 = ml_dtypes.bfloat16
    # pre-swizzled for the SBUF layout [128, FCH, R]: row p holds chunk c's
    # partition-p slice, so the const DMA is one contiguous run per partition
    keff_in = np.ascontiguousarray(
        keff2.T.astype(bf16).reshape(FCH, 128, R).transpose(1, 0, 2).reshape(128, FCH * R)
    )
    vaug_in = np.concatenate([V, np.eye(R, dtype=f32)], axis=1).astype(bf16)  # [R, F+R]
    ebias_in = np.ascontiguousarray(sb2[:, None])                        # [R, 1]
    onescol_in = np.ones((R, 1), bf16)

    Xb = X.astype(bf16)
    in_maps = []
    for c in range(NCORES):
        xt_in = np.ascontiguousarray(Xb[c * TPC : (c + 1) * TPC].T)      # [F, TPC]
        in_maps.append(
            {
                "xt": xt_in,
                "keff": keff_in,
                "vaug": vaug_in,
                "ebias": ebias_in,
                "onescol": onescol_in,
            }
        )
    return in_maps


def run_on_device(in_maps, trace: bool = False):
    nc = _get_nc()
    return run_bass_kernel_spmd(nc, in_maps, core_ids=list(range(NCORES)), trace=trace)


def _unblock(arr, width):
    # arr[sup, p, j, w] holds token sup*SUP + j*SUB + p
    return (
        np.asarray(arr, dtype=np.float32).transpose(0, 2, 1, 3).reshape(TPC, width)
    )


def assemble_outputs(results):
    att = np.concatenate(
        [_unblock(results[c]["att"], F) for c in range(NCORES)], axis=0
    )
    attn = np.concatenate(
        [
            np.asarray(results[c]["attn"], dtype=np.float32)
            .transpose(1, 2, 0, 3)
            .reshape(TPC, R)
            for c in range(NCORES)
        ],
        axis=0,
    )
    return att.reshape(B, D, F), attn.reshape(B, D, R)


def kernel(**inputs):
    in_maps = prepare_in_maps(**inputs)
    res = run_on_device(in_maps, trace=False)
    return assemble_outputs(res.results)


# revision 45
# speedup vs baseline: 1.0086x; 1.0086x over previous
"""AnatomicalAttention TRN2 kernel — 8-core data parallel.

Reference computation (B=4096, D=14, R=7, F=768):
    Q = X @ Wq.T + bq                 # [T, F], T = B*D tokens
    K = E @ Wk.T + bk                 # [R, F]
    V = E @ Wv.T + bv                 # [R, F]
    scores = (Q @ K.T) * scale * w    # [T, R]
    attn = softmax(scores, -1)
    attended = attn @ V               # [T, F]

Key rewrite: Q @ K.T = X @ (K @ Wq).T + bq @ K.T — K is only [7, 768], so the
[768,768] Q projection collapses into a rank-7 contraction with the tiny
precomputed matrix Keff = (K @ Wq) * scale * w.  Per 512-token super-tile:
    scoresT[r, t] = sum_f Keff[r, f] * X[t, f]       (PE, f on partitions)
    expT = exp(scoresT + bias) -> bf16               (ACT, PSUM -> SBUF)
    denom[t, g] = expT_g.T @ ones                    (PE, token-major)
    rcb = 1 / denom                                  (DVE, one batched recip)
    up = expT_g.T @ [V | I7]                         (PE: attended + exp cols)
    [attended | attn] = up * rcb[:, g]               (DVE, PSUM -> SBUF bf16)
The two phases are software-pipelined one super-tile apart so PE runs the
next tile's scores while ACT/DVE normalize the previous one; dummy "warm"
matmuls fill DMA-stall gaps so the PE HAM clock gate stays at 2.4 GHz.

Host passes X transposed ([F, T] feature-major) and in bf16 so DMA traffic is
halved and the contraction dim lands on SBUF partitions with contiguous
descriptors.  Outputs return as bf16 in blocked layouts (upcast/untangled on
host); attn accumulates on-chip and ships as one DMA.  Normalization is f32.
"""

from contextlib import ExitStack

import numpy as np
import ml_dtypes

import concourse.bass as bass
import concourse.tile as tile
from concourse import bacc, mybir
from concourse.bass import ds
from concourse.bass_utils import run_bass_kernel_spmd

B, D, R, F = 4096, 14, 7, 768
NCORES = 8
TOK = B * D                  # 57344 tokens
TPC = TOK // NCORES          # 7168 tokens per core
BLK = 512                    # tokens per DMA block
SUP = 512                    # tokens per score super-tile (fp32 moving-max)
SUB = 128                    # tokens per attended sub-tile (PE stationary max)
NBLK = TPC // BLK            # 7
FCH = F // 128               # 6 contraction chunks

_BF16 = mybir.dt.bfloat16
_F32 = mybir.dt.float32


def _build_nc() -> bass.Bass:
    nc = bacc.Bacc(trn_type="TRN2")

    xt = nc.declare_dram_parameter("xt", [F, TPC], _BF16, isOutput=False)
    keff = nc.declare_dram_parameter("keff", [128, FCH * R], _BF16, isOutput=False)
    # vaug = [V | I7]: attended matmul emits exp@V plus the token-major exp
    # columns used for the attn output, in one PE pass.
    vaug = nc.declare_dram_parameter("vaug", [R, F + R], _BF16, isOutput=False)
    ebias = nc.declare_dram_parameter("ebias", [R, 1], _F32, isOutput=False)
    onescol = nc.declare_dram_parameter("onescol", [R, 1], _BF16, isOutput=False)
    # blocked output layouts (host untangles): att[sup, p, j, f] = token sup*SUP+j*SUB+p
    NSUP = TPC // SUP
    GRP = SUP // SUB
    att = nc.declare_dram_parameter("att", [NSUP, 128, GRP, F], _BF16, isOutput=True)
    attn = nc.declare_dram_parameter("attn", [128, NSUP, GRP, R], _BF16, isOutput=True)

    with tile.TileContext(nc) as tc, ExitStack() as ctx:
        consts = ctx.enter_context(tc.tile_pool(name="consts", bufs=1))
        xpool = ctx.enter_context(tc.tile_pool(name="xpool", bufs=1))
        epool = ctx.enter_context(tc.tile_pool(name="epool", bufs=1))
        rfpool = ctx.enter_context(tc.tile_pool(name="rfpool", bufs=1))
        opool = ctx.enter_context(tc.tile_pool(name="opool", bufs=1))
        psa = ctx.enter_context(tc.tile_pool(name="psa", bufs=1, space="PSUM"))
        psd = ctx.enter_context(tc.tile_pool(name="psd", bufs=1, space="PSUM"))
        psu = ctx.enter_context(tc.tile_pool(name="psu", bufs=1, space="PSUM"))
        psw = ctx.enter_context(tc.tile_pool(name="psw", bufs=1, space="PSUM"))

        keff_sb = consts.tile([128, FCH, R], _BF16)
        nc.sync.dma_start(out=keff_sb, in_=keff[:].rearrange("p (c r) -> p c r", c=FCH))
        vaug_sb = consts.tile([R, F + R], _BF16)
        nc.sync.dma_start(out=vaug_sb, in_=vaug[:])
        ebias_sb = consts.tile([R, 1], _F32)
        nc.sync.dma_start(out=ebias_sb, in_=ebias[:])
        onescol_sb = consts.tile([R, 1], _BF16)
        nc.sync.dma_start(out=onescol_sb, in_=onescol[:])
        # PE keep-warm scratch: dummy matmuls fill DMA-stall gaps so the HAM
        # clock gate never sees an idle window and the PE stays at 2.4 GHz.
        warmsrc = consts.tile([128, 512], _BF16)
        nc.gpsimd.memset(warmsrc[:], 0.0)
        warm_ps = psw.tile([1, 512], _F32, tag="warm")
        # all attn output accumulates here (784B/partition); one DMA at the end
        attn_all = consts.tile([128, NSUP, GRP, R], _BF16)

        def warm(k):
            for _ in range(k):
                nc.tensor.matmul(warm_ps, warmsrc[:, 0:1], warmsrc, start=True, stop=True)

        xt_r = xt[:].rearrange("(c p) t -> p c t", p=128)

        def scores_phase(si):
            """Load + score matmuls.  Returns state."""
            warm(4)
            xts = xpool.tile([128, FCH, SUP], _BF16, tag=f"xts{si % 5}", name="xts")
            nc.sync.dma_start(out=xts, in_=xt_r[:, :, ds(si * SUP, SUP)])
            sct = psa.tile([R, SUP], _F32, tag=f"ps_sc{si % 2}")
            for c in range(FCH):
                nc.tensor.matmul(
                    sct,
                    keff_sb[:, c, :],
                    xts[:, c, :],
                    start=(c == 0),
                    stop=(c == FCH - 1),
                )
            return (si, sct)

        def exp_phase(state):
            """Emitted after the previous super's scales so ACT's in-order
            stream doesn't block them behind this exp."""
            si, sct = state
            expt = epool.tile([R, SUP], _BF16, tag=f"expt{si % 6}")
            nc.scalar.activation(
                expt, sct, mybir.ActivationFunctionType.Exp, bias=ebias_sb, scale=1.0
            )
            return (si, expt)

        def tail_phase(state):
            """Denominators, reciprocal, attended matmuls, scales, store."""
            si, expt = state
            dnt = psd.tile([128, GRP], _F32, tag="ps_dn")
            for sj in range(GRP):
                nc.tensor.matmul(
                    dnt[:, sj : sj + 1], expt[:, ds(sj * SUB, SUB)], onescol_sb
                )
            rcb = rfpool.tile([128, GRP], _F32, tag=f"rcb{si % 3}")
            nc.vector.reciprocal(rcb, dnt)

            atts = opool.tile([128, GRP, F + R], _BF16, tag=f"atts{si % 3}")
            # unnormalized attended (+ token-major exp columns), scaled by
            # 1/denom on the way out of PSUM; scales split across DVE/ACT.
            for sj in range(GRP):
                lhs = expt[:, ds(sj * SUB, SUB)]
                up = psu.tile([128, F + R], _F32, tag=f"up{sj % 2}")
                nc.tensor.matmul(up[:, 0:512], lhs, vaug_sb[:, 0:512])
                nc.tensor.matmul(up[:, 512 : F + R], lhs, vaug_sb[:, 512 : F + R])
                rc = rcb[:, sj : sj + 1]
                if sj < 2:
                    nc.vector.tensor_scalar_mul(atts[:, sj, :], up[:], rc)
                else:
                    nc.scalar.mul(atts[:, sj, :], up[:], mul=rc)
            # gather the attn columns into the whole-kernel accumulator (on
            # GPSIMD, which is otherwise idle); DMA'd once at kernel end.
            nc.gpsimd.tensor_copy(out=attn_all[:, si], in_=atts[:, :, F : F + R])
            nc.gpsimd.dma_start(out=att[si], in_=atts[:, :, 0:F])

        warm(14)  # pre-warm HAM while the first input DMA is in flight
        prev = None
        for si in range(NSUP):
            st = scores_phase(si)
            if prev is not None:
                tail_phase(prev)
            prev = exp_phase(st)
        tail_phase(prev)
        nc.gpsimd.dma_start(out=attn[:], in_=attn_all)

    nc.compile()
    return nc


_NC_CACHE: list = []


def _get_nc() -> bass.Bass:
    if not _NC_CACHE:
        _NC_CACHE.append(_build_nc())
    return _NC_CACHE[0]


def prepare_in_maps(features, region_embeddings, Wq, bq, Wk, bk, Wv, bv, region_weights):
    f32 = np.float32
    X = np.asarray(features, dtype=f32).reshape(TOK, F)
    E = np.asarray(region_embeddings, dtype=f32)
    Wq = np.asarray(Wq, dtype=f32)
    bq = np.asarray(bq, dtype=f32)
    Wk = np.asarray(Wk, dtype=f32)
    bk = np.asarray(bk, dtype=f32)
    Wv = np.asarray(Wv, dtype=f32)
    bv = np.asarray(bv, dtype=f32)
    w = np.asarray(region_weights, dtype=f32)

    scale = f32(F) ** -0.5
    K = E @ Wk.T + bk                      # [R, F]
    V = E @ Wv.T + bv                      # [R, F]
    cw = (scale * w).astype(f32)           # [R]
    keff2 = (K @ Wq) * cw[:, None]         # [R, F]
    sb2 = (K @ bq) * cw                    # [R]

    bf16 = ml_dtypes.bfloat16
    # pre-swizzled for the SBUF layout [128, FCH, R]: row p holds chunk c's
    # partition-p slice, so the const DMA is one contiguous run per partition
    keff_in = np.ascontiguousarray(
        keff2.T.astype(bf16).reshape(FCH, 128, R).transpose(1, 0, 2).reshape(128, FCH * R)
    )
    vaug_in = np.concatenate([V, np.eye(R, dtype=f32)], axis=1).astype(bf16)  # [R, F+R]
    ebias_in = np.ascontiguousarray(sb2[:, None])                        # [R, 1]
    onescol_in = np.ones((R, 1), bf16)

    Xb = X.astype(bf16)
    in_maps = []
    for c in range(NCORES):
        xt_in = np.ascontiguousarray(Xb[c * TPC : (c + 1) * TPC].T)      # [F, TPC]
        in_maps.append(
            {
                "xt": xt_in,
                "keff": keff_in,
                "vaug": vaug_in,
                "ebias": ebias_in,
                "onescol": onescol_in,
            }
        )
    return in_maps


def run_on_device(in_maps, trace: bool = False):
    nc = _get_nc()
    return run_bass_kernel_spmd(nc, in_maps, core_ids=list(range(NCORES)), trace=trace)


def _unblock(arr, width):
    # arr[sup, p, j, w] holds token sup*SUP + j*SUB + p
    return (
        np.asarray(arr, dtype=np.float32).transpose(0, 2, 1, 3).reshape(TPC, width)
    )


def assemble_outputs(results):
    att = np.concatenate(
        [_unblock(results[c]["att"], F) for c in range(NCORES)], axis=0
    )
    attn = np.concatenate(
        [
            np.asarray(results[c]["attn"], dtype=np.float32)
            .transpose(1, 2, 0, 3)
            .reshape(TPC, R)
            for c in range(NCORES)
        ],
        axis=0,
    )
    return att.reshape(B, D, F), attn.reshape(B, D, R)


def kernel(**inputs):
    in_maps = prepare_in_maps(**inputs)
    res = run_on_device(in_maps, trace=False)
    return assemble_outputs(res.results)


# revision 46
# speedup vs baseline: 1.0117x; 1.0031x over previous
"""AnatomicalAttention TRN2 kernel — 8-core data parallel.

Reference computation (B=4096, D=14, R=7, F=768):
    Q = X @ Wq.T + bq                 # [T, F], T = B*D tokens
    K = E @ Wk.T + bk                 # [R, F]
    V = E @ Wv.T + bv                 # [R, F]
    scores = (Q @ K.T) * scale * w    # [T, R]
    attn = softmax(scores, -1)
    attended = attn @ V               # [T, F]

Key rewrite: Q @ K.T = X @ (K @ Wq).T + bq @ K.T — K is only [7, 768], so the
[768,768] Q projection collapses into a rank-7 contraction with the tiny
precomputed matrix Keff = (K @ Wq) * scale * w.  Per 512-token super-tile:
    scoresT[r, t] = sum_f Keff[r, f] * X[t, f]       (PE, f on partitions)
    expT = exp(scoresT + bias) -> bf16               (ACT, PSUM -> SBUF)
    denom[t, g] = expT_g.T @ ones                    (PE, token-major)
    rcb = 1 / denom                                  (DVE, one batched recip)
    up = expT_g.T @ [V | I7]                         (PE: attended + exp cols)
    [attended | attn] = up * rcb[:, g]               (DVE, PSUM -> SBUF bf16)
The two phases are software-pipelined one super-tile apart so PE runs the
next tile's scores while ACT/DVE normalize the previous one; dummy "warm"
matmuls fill DMA-stall gaps so the PE HAM clock gate stays at 2.4 GHz.

Host passes X transposed ([F, T] feature-major) and in bf16 so DMA traffic is
halved and the contraction dim lands on SBUF partitions with contiguous
descriptors.  Outputs return as bf16 in blocked layouts (upcast/untangled on
host); attn accumulates on-chip and ships as one DMA.  Normalization is f32.
"""

from contextlib import ExitStack

import numpy as np
import ml_dtypes

import concourse.bass as bass
import concourse.tile as tile
from concourse import bacc, mybir
from concourse.bass import ds
from concourse.bass_utils import run_bass_kernel_spmd

B, D, R, F = 4096, 14, 7, 768
NCORES = 8
TOK = B * D                  # 57344 tokens
TPC = TOK // NCORES          # 7168 tokens per core
BLK = 512                    # tokens per DMA block
SUP = 512                    # tokens per score super-tile (fp32 moving-max)
SUB = 128                    # tokens per attended sub-tile (PE stationary max)
NBLK = TPC // BLK            # 7
FCH = F // 128               # 6 contraction chunks

_BF16 = mybir.dt.bfloat16
_F32 = mybir.dt.float32


def _build_nc() -> bass.Bass:
    nc = bacc.Bacc(trn_type="TRN2")

    xt = nc.declare_dram_parameter("xt", [F, TPC], _BF16, isOutput=False)
    keff = nc.declare_dram_parameter("keff", [128, FCH * R], _BF16, isOutput=False)
    # vaug = [V | I7]: attended matmul emits exp@V plus the token-major exp
    # columns used for the attn output, in one PE pass.
    vaug = nc.declare_dram_parameter("vaug", [R, F + R], _BF16, isOutput=False)
    ebias = nc.declare_dram_parameter("ebias", [R, 1], _F32, isOutput=False)
    onescol = nc.declare_dram_parameter("onescol", [R, 1], _BF16, isOutput=False)
    # blocked output layouts (host untangles): att[sup, p, j, f] = token sup*SUP+j*SUB+p
    NSUP = TPC // SUP
    GRP = SUP // SUB
    att = nc.declare_dram_parameter("att", [NSUP, 128, GRP, F], _BF16, isOutput=True)
    attn = nc.declare_dram_parameter("attn", [128, NSUP, GRP, R], _BF16, isOutput=True)

    with tile.TileContext(nc) as tc, ExitStack() as ctx:
        consts = ctx.enter_context(tc.tile_pool(name="consts", bufs=1))
        xpool = ctx.enter_context(tc.tile_pool(name="xpool", bufs=1))
        epool = ctx.enter_context(tc.tile_pool(name="epool", bufs=1))
        rfpool = ctx.enter_context(tc.tile_pool(name="rfpool", bufs=1))
        opool = ctx.enter_context(tc.tile_pool(name="opool", bufs=1))
        psa = ctx.enter_context(tc.tile_pool(name="psa", bufs=1, space="PSUM"))
        psd = ctx.enter_context(tc.tile_pool(name="psd", bufs=1, space="PSUM"))
        psu = ctx.enter_context(tc.tile_pool(name="psu", bufs=1, space="PSUM"))
        psw = ctx.enter_context(tc.tile_pool(name="psw", bufs=1, space="PSUM"))

        keff_sb = consts.tile([128, FCH, R], _BF16)
        nc.sync.dma_start(out=keff_sb, in_=keff[:].rearrange("p (c r) -> p c r", c=FCH))
        vaug_sb = consts.tile([R, F + R], _BF16)
        nc.sync.dma_start(out=vaug_sb, in_=vaug[:])
        ebias_sb = consts.tile([R, 1], _F32)
        nc.sync.dma_start(out=ebias_sb, in_=ebias[:])
        onescol_sb = consts.tile([R, 1], _BF16)
        nc.sync.dma_start(out=onescol_sb, in_=onescol[:])
        # PE keep-warm scratch: dummy matmuls fill DMA-stall gaps so the HAM
        # clock gate never sees an idle window and the PE stays at 2.4 GHz.
        warmsrc = consts.tile([128, 512], _BF16)
        nc.gpsimd.memset(warmsrc[:], 0.0)
        warm_ps = psw.tile([1, 512], _F32, tag="warm")
        # all attn output accumulates here (784B/partition); one DMA at the end
        attn_all = consts.tile([128, NSUP, GRP, R], _BF16)

        def warm(k):
            for _ in range(k):
                nc.tensor.matmul(warm_ps, warmsrc[:, 0:1], warmsrc, start=True, stop=True)

        xt_r = xt[:].rearrange("(c p) t -> p c t", p=128)

        def scores_phase(si):
            """Load + score matmuls.  Returns state."""
            warm(4)
            xts = xpool.tile([128, FCH, SUP], _BF16, tag=f"xts{si % 5}", name="xts")
            nc.sync.dma_start(out=xts, in_=xt_r[:, :, ds(si * SUP, SUP)])
            sct = psa.tile([R, SUP], _F32, tag=f"ps_sc{si % 2}")
            for c in range(FCH):
                nc.tensor.matmul(
                    sct,
                    keff_sb[:, c, :],
                    xts[:, c, :],
                    start=(c == 0),
                    stop=(c == FCH - 1),
                )
            return (si, sct)

        def exp_phase(state):
            """Emitted after the previous super's scales so ACT's in-order
            stream doesn't block them behind this exp."""
            si, sct = state
            expt = epool.tile([R, SUP], _BF16, tag=f"expt{si % 6}")
            nc.scalar.activation(
                expt, sct, mybir.ActivationFunctionType.Exp, bias=ebias_sb, scale=1.0
            )
            return (si, expt)

        def tail_phase(state):
            """Denominators, reciprocal, attended matmuls, scales, store."""
            si, expt = state
            dnt = psd.tile([128, GRP], _F32, tag="ps_dn")
            for sj in range(GRP):
                nc.tensor.matmul(
                    dnt[:, sj : sj + 1], expt[:, ds(sj * SUB, SUB)], onescol_sb
                )
            rcb = rfpool.tile([128, GRP], _F32, tag=f"rcb{si % 3}")
            nc.vector.reciprocal(rcb, dnt)

            atts = opool.tile([128, GRP, F + R], _BF16, tag=f"atts{si % 3}")
            # unnormalized attended (+ token-major exp columns), scaled by
            # 1/denom on the way out of PSUM; scales split across DVE/ACT.
            for sj in range(GRP):
                lhs = expt[:, ds(sj * SUB, SUB)]
                up = psu.tile([128, F + R], _F32, tag=f"up{sj % 2}")
                nc.tensor.matmul(up[:, 0:512], lhs, vaug_sb[:, 0:512])
                nc.tensor.matmul(up[:, 512 : F + R], lhs, vaug_sb[:, 512 : F + R])
                rc = rcb[:, sj : sj + 1]
                if sj < 2:
                    nc.vector.tensor_scalar_mul(atts[:, sj, :], up[:], rc)
                else:
                    nc.scalar.mul(atts[:, sj, :], up[:], mul=rc)
            # gather the attn columns into the whole-kernel accumulator (on
            # GPSIMD, which is otherwise idle); DMA'd once at kernel end.
            nc.gpsimd.tensor_copy(out=attn_all[:, si], in_=atts[:, :, F : F + R])
            nc.gpsimd.dma_start(out=att[si], in_=atts[:, :, 0:F])

        # pre-warm the HAM while the first input DMA is in flight: 18 cold
        # dummies = ~7.7us of continuous PE busy, covering a full 3.4us
        # activity window at any phase of the free-running HAM counter
        warm(18)
        prev = None
        for si in range(NSUP):
            st = scores_phase(si)
            if prev is not None:
                tail_phase(prev)
            prev = exp_phase(st)
        tail_phase(prev)
        nc.gpsimd.dma_start(out=attn[:], in_=attn_all)

    nc.compile()
    return nc


_NC_CACHE: list = []


def _get_nc() -> bass.Bass:
    if not _NC_CACHE:
        _NC_CACHE.append(_build_nc())
    return _NC_CACHE[0]


def prepare_in_maps(features, region_embeddings, Wq, bq, Wk, bk, Wv, bv, region_weights):
    f32 = np.float32
    X = np.asarray(features, dtype=f32).reshape(TOK, F)
    E = np.asarray(region_embeddings, dtype=f32)
    Wq = np.asarray(Wq, dtype=f32)
    bq = np.asarray(bq, dtype=f32)
    Wk = np.asarray(Wk, dtype=f32)
    bk = np.asarray(bk, dtype=f32)
    Wv = np.asarray(Wv, dtype=f32)
    bv = np.asarray(bv, dtype=f32)
    w = np.asarray(region_weights, dtype=f32)

    scale = f32(F) ** -0.5
    K = E @ Wk.T + bk                      # [R, F]
    V = E @ Wv.T + bv                      # [R, F]
    cw = (scale * w).astype(f32)           # [R]
    keff2 = (K @ Wq) * cw[:, None]         # [R, F]
    sb2 = (K @ bq) * cw                    # [R]

    bf16 = ml_dtypes.bfloat16
    # pre-swizzled for the SBUF layout [128, FCH, R]: row p holds chunk c's
    # partition-p slice, so the const DMA is one contiguous run per partition
    keff_in = np.ascontiguousarray(
        keff2.T.astype(bf16).reshape(FCH, 128, R).transpose(1, 0, 2).reshape(128, FCH * R)
    )
    vaug_in = np.concatenate([V, np.eye(R, dtype=f32)], axis=1).astype(bf16)  # [R, F+R]
    ebias_in = np.ascontiguousarray(sb2[:, None])                        # [R, 1]
    onescol_in = np.ones((R, 1), bf16)

    Xb = X.astype(bf16)
    in_maps = []
    for c in range(NCORES):
        xt_in = np.ascontiguousarray(Xb[c * TPC : (c + 1) * TPC].T)      # [F, TPC]
        in_maps.append(
            {
                "xt": xt_in,
                "keff": keff_in,
                "vaug": vaug_in,
                "ebias": ebias_in,
                "onescol": onescol_in,
            }
        )
    return in_maps


def run_on_device(in_maps, trace: bool = False):
    nc = _get_nc()
    return run_bass_kernel_spmd(nc, in_maps, core_ids=list(range(NCORES)), trace=trace)


def _unblock(arr, width):
    # arr[sup, p, j, w] holds token sup*SUP + j*SUB + p
    return (
        np.asarray(arr, dtype=np.float32).transpose(0, 2, 1, 3).reshape(TPC, width)
    )


def assemble_outputs(results):
    att = np.concatenate(
        [_unblock(results[c]["att"], F) for c in range(NCORES)], axis=0
    )
    attn = np.concatenate(
        [
            np.asarray(results[c]["attn"], dtype=np.float32)
            .transpose(1, 2, 0, 3)
            .reshape(TPC, R)
            for c in range(NCORES)
        ],
        axis=0,
    )
    return att.reshape(B, D, F), attn.reshape(B, D, R)


def kernel(**inputs):
    in_maps = prepare_in_maps(**inputs)
    res = run_on_device(in_maps, trace=False)
    return assemble_outputs(res.results)


# revision 47
# speedup vs baseline: 1.1829x; 1.1692x over previous
"""AnatomicalAttention TRN2 kernel — 8-core data parallel.

Reference computation (B=4096, D=14, R=7, F=768):
    Q = X @ Wq.T + bq                 # [T, F], T = B*D tokens
    K = E @ Wk.T + bk                 # [R, F]
    V = E @ Wv.T + bv                 # [R, F]
    scores = (Q @ K.T) * scale * w    # [T, R]
    attn = softmax(scores, -1)
    attended = attn @ V               # [T, F]

Key rewrite: Q @ K.T = X @ (K @ Wq).T + bq @ K.T — K is only [7, 768], so the
[768,768] Q projection collapses into a rank-7 contraction with the tiny
precomputed matrix Keff = (K @ Wq) * scale * w.  Per 512-token super-tile:
    scoresT[r, t] = sum_f Keff[r, f] * X[t, f]       (PE, f on partitions)
    expT = exp(scoresT + bias) -> bf16               (ACT, PSUM -> SBUF)
    denom[t, g] = expT_g.T @ ones                    (PE, token-major)
    rcb = 1 / denom                                  (DVE, one batched recip)
    up = expT_g.T @ [V | I7]                         (PE: attended + exp cols)
    [attended | attn] = up * rcb[:, g]               (DVE, PSUM -> SBUF bf16)
The two phases are software-pipelined one super-tile apart so PE runs the
next tile's scores while ACT/DVE normalize the previous one; dummy "warm"
matmuls fill DMA-stall gaps so the PE HAM clock gate stays at 2.4 GHz.

Host passes X transposed ([F, T] feature-major) and in bf16 so DMA traffic is
halved and the contraction dim lands on SBUF partitions with contiguous
descriptors.  Outputs return as bf16 in blocked layouts (upcast/untangled on
host); attn accumulates on-chip and ships as one DMA.  Normalization is f32.
"""

from contextlib import ExitStack

import numpy as np
import ml_dtypes

import concourse.bass as bass
import concourse.tile as tile
from concourse import bacc, mybir
from concourse.bass import ds
from concourse.bass_utils import run_bass_kernel_spmd

B, D, R, F = 4096, 14, 7, 768
NCORES = 8
TOK = B * D                  # 57344 tokens
TPC = TOK // NCORES          # 7168 tokens per core
BLK = 512                    # tokens per DMA block
SUP = 512                    # tokens per score super-tile (fp32 moving-max)
SUB = 128                    # tokens per attended sub-tile (PE stationary max)
NBLK = TPC // BLK            # 7
FCH = F // 128               # 6 contraction chunks

_BF16 = mybir.dt.bfloat16
_F32 = mybir.dt.float32


def _build_nc() -> bass.Bass:
    nc = bacc.Bacc(trn_type="TRN2")

    xt = nc.declare_dram_parameter("xt", [F, TPC], _BF16, isOutput=False)
    keff = nc.declare_dram_parameter("keff", [128, FCH * R], _BF16, isOutput=False)
    # vaug = [V | I7]: attended matmul emits exp@V plus the token-major exp
    # columns used for the attn output, in one PE pass.
    vaug = nc.declare_dram_parameter("vaug", [R, F + R], _BF16, isOutput=False)
    ebias = nc.declare_dram_parameter("ebias", [R, 1], _F32, isOutput=False)
    onescol = nc.declare_dram_parameter("onescol", [R, 1], _BF16, isOutput=False)
    # blocked output layouts (host untangles): att[sup, p, j, f] = token sup*SUP+j*SUB+p
    NSUP = TPC // SUP
    GRP = SUP // SUB
    att = nc.declare_dram_parameter("att", [NSUP, 128, GRP, F], _BF16, isOutput=True)
    attn = nc.declare_dram_parameter("attn", [128, NSUP, GRP, R], _BF16, isOutput=True)

    with tile.TileContext(nc) as tc, ExitStack() as ctx:
        consts = ctx.enter_context(tc.tile_pool(name="consts", bufs=1))
        xpool = ctx.enter_context(tc.tile_pool(name="xpool", bufs=1))
        epool = ctx.enter_context(tc.tile_pool(name="epool", bufs=1))
        rfpool = ctx.enter_context(tc.tile_pool(name="rfpool", bufs=1))
        opool = ctx.enter_context(tc.tile_pool(name="opool", bufs=1))
        psa = ctx.enter_context(tc.tile_pool(name="psa", bufs=1, space="PSUM"))
        psd = ctx.enter_context(tc.tile_pool(name="psd", bufs=1, space="PSUM"))
        psu = ctx.enter_context(tc.tile_pool(name="psu", bufs=1, space="PSUM"))
        psw = ctx.enter_context(tc.tile_pool(name="psw", bufs=1, space="PSUM"))

        keff_sb = consts.tile([128, FCH, R], _BF16)
        nc.sync.dma_start(out=keff_sb, in_=keff[:].rearrange("p (c r) -> p c r", c=FCH))
        vaug_sb = consts.tile([R, F + R], _BF16)
        nc.sync.dma_start(out=vaug_sb, in_=vaug[:])
        ebias_sb = consts.tile([R, 1], _F32)
        nc.sync.dma_start(out=ebias_sb, in_=ebias[:])
        onescol_sb = consts.tile([R, 1], _BF16)
        nc.sync.dma_start(out=onescol_sb, in_=onescol[:])
        # PE keep-warm scratch: dummy matmuls fill DMA-stall gaps so the HAM
        # clock gate never sees an idle window and the PE stays at 2.4 GHz.
        warmsrc = consts.tile([128, 512], _BF16)
        nc.gpsimd.memset(warmsrc[:], 0.0)
        warm_ps = psw.tile([1, 512], _F32, tag="warm")
        # all attn output accumulates here (784B/partition); one DMA at the end
        attn_all = consts.tile([128, NSUP, GRP, R], _BF16)

        def warm(k):
            for _ in range(k):
                nc.tensor.matmul(warm_ps, warmsrc[:, 0:1], warmsrc, start=True, stop=True)

        xt_r = xt[:].rearrange("(c p) t -> p c t", p=128)

        def scores_phase(si):
            """Load + score matmuls.  Returns state."""
            warm(4)
            xts = xpool.tile([128, FCH, SUP], _BF16, tag=f"xts{si % 5}", name="xts")
            nc.sync.dma_start(out=xts, in_=xt_r[:, :, ds(si * SUP, SUP)])
            sct = psa.tile([R, SUP], _F32, tag=f"ps_sc{si % 2}")
            for c in range(FCH):
                nc.tensor.matmul(
                    sct,
                    keff_sb[:, c, :],
                    xts[:, c, :],
                    start=(c == 0),
                    stop=(c == FCH - 1),
                )
            return (si, sct)

        def exp_phase(state):
            """Emitted after the previous super's scales so ACT's in-order
            stream doesn't block them behind this exp."""
            si, sct = state
            expt = epool.tile([R, SUP], _BF16, tag=f"expt{si % 6}")
            nc.scalar.activation(
                expt, sct, mybir.ActivationFunctionType.Exp, bias=ebias_sb, scale=1.0
            )
            return (si, expt)

        def tail_phase(state):
            """Denominators, reciprocal, attended matmuls, scales, store."""
            si, expt = state
            dnt = psd.tile([128, GRP], _F32, tag="ps_dn")
            for sj in range(GRP):
                nc.tensor.matmul(
                    dnt[:, sj : sj + 1], expt[:, ds(sj * SUB, SUB)], onescol_sb
                )
            rcb = rfpool.tile([128, GRP], _F32, tag=f"rcb{si % 3}")
            nc.vector.reciprocal(rcb, dnt)

            atts = opool.tile([128, GRP, F + R], _BF16, tag=f"atts{si % 3}")
            # unnormalized attended (+ token-major exp columns), scaled by
            # 1/denom on the way out of PSUM; scales split across DVE/ACT.
            for sj in range(GRP):
                lhs = expt[:, ds(sj * SUB, SUB)]
                up = psu.tile([128, F + R], _F32, tag=f"up{sj % 2}")
                nc.tensor.matmul(up[:, 0:512], lhs, vaug_sb[:, 0:512])
                nc.tensor.matmul(up[:, 512 : F + R], lhs, vaug_sb[:, 512 : F + R])
                rc = rcb[:, sj : sj + 1]
                if sj < 2:
                    nc.vector.tensor_scalar_mul(atts[:, sj, :], up[:], rc)
                else:
                    nc.scalar.mul(atts[:, sj, :], up[:], mul=rc)
            # gather the attn columns into the whole-kernel accumulator (on
            # GPSIMD, which is otherwise idle); DMA'd once at kernel end.
            nc.gpsimd.tensor_copy(out=attn_all[:, si], in_=atts[:, :, F : F + R])
            nc.gpsimd.dma_start(out=att[si], in_=atts[:, :, 0:F])

        warm(14)  # pre-warm HAM while the first input DMA is in flight
        prev = None
        for si in range(NSUP):
            st = scores_phase(si)
            if prev is not None:
                tail_phase(prev)
            prev = exp_phase(st)
        tail_phase(prev)
        nc.gpsimd.dma_start(out=attn[:], in_=attn_all)

    nc.compile()
    return nc


_NC_CACHE: list = []


def _get_nc() -> bass.Bass:
    if not _NC_CACHE:
        _NC_CACHE.append(_build_nc())
    return _NC_CACHE[0]


def prepare_in_maps(features, region_embeddings, Wq, bq, Wk, bk, Wv, bv, region_weights):
    f32 = np.float32
    X = np.asarray(features, dtype=f32).reshape(TOK, F)
    E = np.asarray(region_embeddings, dtype=f32)
    Wq = np.asarray(Wq, dtype=f32)
    bq = np.asarray(bq, dtype=f32)
    Wk = np.asarray(Wk, dtype=f32)
    bk = np.asarray(bk, dtype=f32)
    Wv = np.asarray(Wv, dtype=f32)
    bv = np.asarray(bv, dtype=f32)
    w = np.asarray(region_weights, dtype=f32)

    scale = f32(F) ** -0.5
    K = E @ Wk.T + bk                      # [R, F]
    V = E @ Wv.T + bv                      # [R, F]
    cw = (scale * w).astype(f32)           # [R]
    keff2 = (K @ Wq) * cw[:, None]         # [R, F]
    sb2 = (K @ bq) * cw                    # [R]

    bf16 = ml_dtypes.bfloat16
    # pre-swizzled for the SBUF layout [128, FCH, R]: row p holds chunk c's
    # partition-p slice, so the const DMA is one contiguous run per partition
    keff_in = np.ascontiguousarray(
        keff2.T.astype(bf16).reshape(FCH, 128, R).transpose(1, 0, 2).reshape(128, FCH * R)
    )
    vaug_in = np.concatenate([V, np.eye(R, dtype=f32)], axis=1).astype(bf16)  # [R, F+R]
    ebias_in = np.ascontiguousarray(sb2[:, None])                        # [R, 1]
    onescol_in = np.ones((R, 1), bf16)

    Xb = X.astype(bf16)
    in_maps = []
    for c in range(NCORES):
        xt_in = np.ascontiguousarray(Xb[c * TPC : (c + 1) * TPC].T)      # [F, TPC]
        in_maps.append(
            {
                "xt": xt_in,
                "keff": keff_in,
                "vaug": vaug_in,
                "ebias": ebias_in,
                "onescol": onescol_in,
            }
        )
    return in_maps


def run_on_device(in_maps, trace: bool = False):
    nc = _get_nc()
    return run_bass_kernel_spmd(nc, in_maps, core_ids=list(range(NCORES)), trace=trace)


def _unblock(arr, width):
    # arr[sup, p, j, w] holds token sup*SUP + j*SUB + p
    return (
        np.asarray(arr, dtype=np.float32).transpose(0, 2, 1, 3).reshape(TPC, width)
    )


def assemble_outputs(results):
    att = np.concatenate(
        [_unblock(results[c]["att"], F) for c in range(NCORES)], axis=0
    )
    attn = np.concatenate(
        [
            np.asarray(results[c]["attn"], dtype=np.float32)
            .transpose(1, 2, 0, 3)
            .reshape(TPC, R)
            for c in range(NCORES)
        ],
        axis=0,
    )
    return att.reshape(B, D, F), attn.reshape(B, D, R)


def kernel(**inputs):
    in_maps = prepare_in_maps(**inputs)
    res = run_on_device(in_maps, trace=False)
    return assemble_outputs(res.results)


# revision 48
# speedup vs baseline: 1.2033x; 1.0172x over previous
"""AnatomicalAttention TRN2 kernel — 8-core data parallel.

Reference computation (B=4096, D=14, R=7, F=768):
    Q = X @ Wq.T + bq                 # [T, F], T = B*D tokens
    K = E @ Wk.T + bk                 # [R, F]
    V = E @ Wv.T + bv                 # [R, F]
    scores = (Q @ K.T) * scale * w    # [T, R]
    attn = softmax(scores, -1)
    attended = attn @ V               # [T, F]

Key rewrite: Q @ K.T = X @ (K @ Wq).T + bq @ K.T — K is only [7, 768], so the
[768,768] Q projection collapses into a rank-7 contraction with the tiny
precomputed matrix Keff = (K @ Wq) * scale * w.  Per 512-token super-tile:
    scoresT[r, t] = sum_f Keff[r, f] * X[t, f]       (PE, f on partitions)
    expT = exp(scoresT + bias) -> bf16               (ACT, PSUM -> SBUF)
    denom[t, g] = expT_g.T @ ones                    (PE, token-major)
    rcb = 1 / denom                                  (DVE, one batched recip)
    up = expT_g.T @ [V | I7]                         (PE: attended + exp cols)
    [attended | attn] = up * rcb[:, g]               (DVE, PSUM -> SBUF bf16)
The two phases are software-pipelined one super-tile apart so PE runs the
next tile's scores while ACT/DVE normalize the previous one; dummy "warm"
matmuls fill DMA-stall gaps so the PE HAM clock gate stays at 2.4 GHz.

Host passes X transposed ([F, T] feature-major) and in bf16 so DMA traffic is
halved and the contraction dim lands on SBUF partitions with contiguous
descriptors.  Outputs return as bf16 in blocked layouts (upcast/untangled on
host); attn accumulates on-chip and ships as one DMA.  Normalization is f32.
"""

from contextlib import ExitStack

import numpy as np
import ml_dtypes

import concourse.bass as bass
import concourse.tile as tile
from concourse import bacc, mybir
from concourse.bass import ds
from concourse.bass_utils import run_bass_kernel_spmd

B, D, R, F = 4096, 14, 7, 768
NCORES = 8
TOK = B * D                  # 57344 tokens
TPC = TOK // NCORES          # 7168 tokens per core
BLK = 512                    # tokens per DMA block
SUP = 512                    # tokens per score super-tile (fp32 moving-max)
SUB = 128                    # tokens per attended sub-tile (PE stationary max)
NBLK = TPC // BLK            # 7
FCH = F // 128               # 6 contraction chunks

_BF16 = mybir.dt.bfloat16
_F32 = mybir.dt.float32


def _build_nc() -> bass.Bass:
    nc = bacc.Bacc(trn_type="TRN2")

    xt = nc.declare_dram_parameter("xt", [F, TPC], _BF16, isOutput=False)
    keff = nc.declare_dram_parameter("keff", [128, FCH * R], _BF16, isOutput=False)
    # vaug = [V | I7]: attended matmul emits exp@V plus the token-major exp
    # columns used for the attn output, in one PE pass.
    vaug = nc.declare_dram_parameter("vaug", [R, F + R], _BF16, isOutput=False)
    ebias = nc.declare_dram_parameter("ebias", [R, 1], _F32, isOutput=False)
    onescol = nc.declare_dram_parameter("onescol", [R, 1], _BF16, isOutput=False)
    # blocked output layouts (host untangles): att[sup, p, j, f] = token sup*SUP+j*SUB+p
    NSUP = TPC // SUP
    GRP = SUP // SUB
    att = nc.declare_dram_parameter("att", [NSUP, 128, GRP, F], _BF16, isOutput=True)
    attn = nc.declare_dram_parameter("attn", [128, NSUP, GRP, R], _BF16, isOutput=True)

    with tile.TileContext(nc) as tc, ExitStack() as ctx:
        consts = ctx.enter_context(tc.tile_pool(name="consts", bufs=1))
        xpool = ctx.enter_context(tc.tile_pool(name="xpool", bufs=1))
        epool = ctx.enter_context(tc.tile_pool(name="epool", bufs=1))
        rfpool = ctx.enter_context(tc.tile_pool(name="rfpool", bufs=1))
        opool = ctx.enter_context(tc.tile_pool(name="opool", bufs=1))
        psa = ctx.enter_context(tc.tile_pool(name="psa", bufs=1, space="PSUM"))
        psd = ctx.enter_context(tc.tile_pool(name="psd", bufs=1, space="PSUM"))
        psu = ctx.enter_context(tc.tile_pool(name="psu", bufs=1, space="PSUM"))
        psw = ctx.enter_context(tc.tile_pool(name="psw", bufs=1, space="PSUM"))

        keff_sb = consts.tile([128, FCH, R], _BF16)
        nc.sync.dma_start(out=keff_sb, in_=keff[:].rearrange("p (c r) -> p c r", c=FCH))
        # first input block issues right after keff, ahead of the small consts,
        # so its 1.4us of SP descriptor-gen and the HBM read start immediately
        xts_pre0 = xpool.tile([128, FCH, SUP], _BF16, tag="xts0", name="xts_pre0")
        nc.sync.dma_start(
            out=xts_pre0,
            in_=xt[:].rearrange("(c p) t -> p c t", p=128)[:, :, ds(0, SUP)],
        )
        vaug_sb = consts.tile([R, F + R], _BF16)
        nc.sync.dma_start(out=vaug_sb, in_=vaug[:])
        ebias_sb = consts.tile([R, 1], _F32)
        nc.sync.dma_start(out=ebias_sb, in_=ebias[:])
        onescol_sb = consts.tile([R, 1], _BF16)
        nc.sync.dma_start(out=onescol_sb, in_=onescol[:])
        # PE keep-warm scratch: dummy matmuls fill DMA-stall gaps so the HAM
        # clock gate never sees an idle window and the PE stays at 2.4 GHz.
        warmsrc = consts.tile([128, 512], _BF16)
        nc.gpsimd.memset(warmsrc[:], 0.0)
        warm_ps = psw.tile([1, 512], _F32, tag="warm")
        # all attn output accumulates here (784B/partition); one DMA at the end
        attn_all = consts.tile([128, NSUP, GRP, R], _BF16)

        def warm(k):
            for _ in range(k):
                nc.tensor.matmul(warm_ps, warmsrc[:, 0:1], warmsrc, start=True, stop=True)

        xt_r = xt[:].rearrange("(c p) t -> p c t", p=128)

        def scores_phase(si):
            """Load + score matmuls.  Returns state."""
            warm(4)
            if si == 0:
                xts = xts_pre0
            else:
                xts = xpool.tile([128, FCH, SUP], _BF16, tag=f"xts{si % 5}", name="xts")
                nc.sync.dma_start(out=xts, in_=xt_r[:, :, ds(si * SUP, SUP)])
            sct = psa.tile([R, SUP], _F32, tag=f"ps_sc{si % 2}")
            for c in range(FCH):
                nc.tensor.matmul(
                    sct,
                    keff_sb[:, c, :],
                    xts[:, c, :],
                    start=(c == 0),
                    stop=(c == FCH - 1),
                )
            return (si, sct)

        def exp_phase(state):
            """Emitted after the previous super's scales so ACT's in-order
            stream doesn't block them behind this exp."""
            si, sct = state
            expt = epool.tile([R, SUP], _BF16, tag=f"expt{si % 6}")
            nc.scalar.activation(
                expt, sct, mybir.ActivationFunctionType.Exp, bias=ebias_sb, scale=1.0
            )
            return (si, expt)

        def tail_phase(state):
            """Denominators, reciprocal, attended matmuls, scales, store."""
            si, expt = state
            dnt = psd.tile([128, GRP], _F32, tag="ps_dn")
            for sj in range(GRP):
                nc.tensor.matmul(
                    dnt[:, sj : sj + 1], expt[:, ds(sj * SUB, SUB)], onescol_sb
                )
            rcb = rfpool.tile([128, GRP], _F32, tag=f"rcb{si % 3}")
            nc.vector.reciprocal(rcb, dnt)

            atts = opool.tile([128, GRP, F + R], _BF16, tag=f"atts{si % 3}")
            # unnormalized attended (+ token-major exp columns), scaled by
            # 1/denom on the way out of PSUM; scales split across DVE/ACT.
            for sj in range(GRP):
                lhs = expt[:, ds(sj * SUB, SUB)]
                up = psu.tile([128, F + R], _F32, tag=f"up{sj % 2}")
                nc.tensor.matmul(up[:, 0:512], lhs, vaug_sb[:, 0:512])
                nc.tensor.matmul(up[:, 512 : F + R], lhs, vaug_sb[:, 512 : F + R])
                rc = rcb[:, sj : sj + 1]
                if sj < 2:
                    nc.vector.tensor_scalar_mul(atts[:, sj, :], up[:], rc)
                else:
                    nc.scalar.mul(atts[:, sj, :], up[:], mul=rc)
            # gather the attn columns into the whole-kernel accumulator (on
            # GPSIMD, which is otherwise idle); DMA'd once at kernel end.
            nc.gpsimd.tensor_copy(out=attn_all[:, si], in_=atts[:, :, F : F + R])
            nc.gpsimd.dma_start(out=att[si], in_=atts[:, :, 0:F])

        warm(14)  # pre-warm HAM while the first input DMA is in flight
        prev = None
        for si in range(NSUP):
            st = scores_phase(si)
            if prev is not None:
                tail_phase(prev)
            prev = exp_phase(st)
        tail_phase(prev)
        nc.gpsimd.dma_start(out=attn[:], in_=attn_all)

    nc.compile()
    return nc


_NC_CACHE: list = []


def _get_nc() -> bass.Bass:
    if not _NC_CACHE:
        _NC_CACHE.append(_build_nc())
    return _NC_CACHE[0]


def prepare_in_maps(features, region_embeddings, Wq, bq, Wk, bk, Wv, bv, region_weights):
    f32 = np.float32
    X = np.asarray(features, dtype=f32).reshape(TOK, F)
    E = np.asarray(region_embeddings, dtype=f32)
    Wq = np.asarray(Wq, dtype=f32)
    bq = np.asarray(bq, dtype=f32)
    Wk = np.asarray(Wk, dtype=f32)
    bk = np.asarray(bk, dtype=f32)
    Wv = np.asarray(Wv, dtype=f32)
    bv = np.asarray(bv, dtype=f32)
    w = np.asarray(region_weights, dtype=f32)

    scale = f32(F) ** -0.5
    K = E @ Wk.T + bk                      # [R, F]
    V = E @ Wv.T + bv                      # [R, F]
    cw = (scale * w).astype(f32)           # [R]
    keff2 = (K @ Wq) * cw[:, None]         # [R, F]
    sb2 = (K @ bq) * cw                    # [R]

    bf16 = ml_dtypes.bfloat16
    # pre-swizzled for the SBUF layout [128, FCH, R]: row p holds chunk c's
    # partition-p slice, so the const DMA is one contiguous run per partition
    keff_in = np.ascontiguousarray(
        keff2.T.astype(bf16).reshape(FCH, 128, R).transpose(1, 0, 2).reshape(128, FCH * R)
    )
    vaug_in = np.concatenate([V, np.eye(R, dtype=f32)], axis=1).astype(bf16)  # [R, F+R]
    ebias_in = np.ascontiguousarray(sb2[:, None])                        # [R, 1]
    onescol_in = np.ones((R, 1), bf16)

    Xb = X.astype(bf16)
    in_maps = []
    for c in range(NCORES):
        xt_in = np.ascontiguousarray(Xb[c * TPC : (c + 1) * TPC].T)      # [F, TPC]
        in_maps.append(
            {
                "xt": xt_in,
                "keff": keff_in,
                "vaug": vaug_in,
                "ebias": ebias_in,
                "onescol": onescol_in,
            }
        )
    return in_maps


def run_on_device(in_maps, trace: bool = False):
    nc = _get_nc()
    return run_bass_kernel_spmd(nc, in_maps, core_ids=list(range(NCORES)), trace=trace)


def _unblock(arr, width):
    # arr[sup, p, j, w] holds token sup*SUP + j*SUB + p
    return (
        np.asarray(arr, dtype=np.float32).transpose(0, 2, 1, 3).reshape(TPC, width)
    )


def assemble_outputs(results):
    att = np.concatenate(
        [_unblock(results[c]["att"], F) for c in range(NCORES)], axis=0
    )
    attn = np.concatenate(
        [
            np.asarray(results[c]["attn"], dtype=np.float32)
            .transpose(1, 2, 0, 3)
            .reshape(TPC, R)
            for c in range(NCORES)
        ],
        axis=0,
    )
    return att.reshape(B, D, F), attn.reshape(B, D, R)


def kernel(**inputs):
    in_maps = prepare_in_maps(**inputs)
    res = run_on_device(in_maps, trace=False)
    return assemble_outputs(res.results)
